# revision 9
# baseline (speedup 1.0000x reference)
"""Causal self-attention on 8 TRN2 NeuronCores — v2.

Reference computation (B=4, T=2048, C=1024, H=16, D=64, fp32):
    qkv = x @ W_attn + b_attn ; split q,k,v ; per-head causal softmax(q k^T / 8) @ v
    y = heads @ W_proj + b_proj

Sharding: core c handles batch b = c//2 and head-half hh = c%2 (8 heads).
QKV weights column-split, W_proj row-split; host sums the two partial
projections per batch and adds the constant bias (b_proj + b_v @ W_proj,
exact because softmax rows sum to 1). No collectives.

v2 design notes (cost-model driven; matmul cost = out_free x cycles/row,
fp8e4+DoubleRow = 0.5 cycles/row regardless of contraction size):
  - q,k generated with fp8 DoubleRow matmuls (x8/wq8/wk8 fp8, weights
    pre-scaled x32 so W~0.02 escapes fp8 subnormals; biases x32 to match;
    the 1/1024 descale is folded into the exp scale).
  - v generated in bf16 (value path dominates the error budget).
  - QK^T uses "padded DoubleRow": per head the contraction is d=64, so
    operands are [64 partitions, 2 k-subtiles, N] with subtile 1 an
    all-zero plane (host-DMA'd zeros) -> 0.5 cycles/row at K=64.
  - Causal mask is folded into PSUM before exp via a rank-127 constant
    bf16 matmul that adds -3e7 above the diagonal; exp then flushes those
    to 0. No DVE masking anywhere.
  - exp runs on chunk PAIRS ([128, 2048] ACT ops over a 4-bank PSUM tile)
    to amortize the ACT access-latency; per-chunk q-trimming via a 4D AP.
  - AV is computed in the SWAPPED layout: stationary = exp-scores chunk
    [kt, q], moving = v_aug [kt, 64 v | 1] -> out y[q, d|den] with free
    size 65 (half the cost of the yT layout), denominator per PARTITION.
    Normalize is then a cheap per-partition tensor_scalar; yT for the
    projection is rebuilt with identity matmul transposes (head B lands
    at partition offset 64 via tile_position).
  - PSUM→SBUF traffic and copies are spread across DVE and Pool (GPSIMD)
    to keep everything under the ACT exp floor (~131 us), the binding
    engine for this kernel.
"""

import numpy as np
import ml_dtypes

import concourse.bacc as bacc
import concourse.mybir as mybir
import concourse.tile as tile
from concourse.bass_utils import run_bass_kernel_spmd

F32 = mybir.dt.float32
BF16 = mybir.dt.bfloat16
FP8 = mybir.dt.float8e4
AF = mybir.ActivationFunctionType
ALU = mybir.AluOpType
DR = mybir.MatmulPerfMode.DoubleRow

N_CORES = 8
B, T, C = 4, 2048, 1024
H, D = 16, 64
CH = 512            # features per core (8 heads * 64)
NFO = 4             # head-pair chunks of 128 features
NTQ = 4             # t quarters of 512
NTC = 16            # t chunks of 128
WS = 32.0           # host-side weight prescale for fp8
EXP_SCALE = 0.125 / (WS * WS)   # softmax 1/8, undoing q,k x32 scales
MBIG = -3.0e7       # causal mask addend (pre-scale): exp -> 0

_cached = {}


def _build_nc():
    nc = bacc.Bacc("TRN2", debug=False, num_devices=N_CORES)

    d_xT = nc.dram_tensor("xT", [C, T], BF16, kind="ExternalInput")
    d_x8 = nc.dram_tensor("x8", [C, T], FP8, kind="ExternalInput")
    d_wq8 = nc.dram_tensor("wq8", [C, CH], FP8, kind="ExternalInput")
    d_wk8 = nc.dram_tensor("wk8", [C, CH], FP8, kind="ExternalInput")
    d_wv = nc.dram_tensor("wv", [C, CH], BF16, kind="ExternalInput")
    d_bq = nc.dram_tensor("bq", [128, NFO], F32, kind="ExternalInput")
    d_bk = nc.dram_tensor("bk", [128, NFO], F32, kind="ExternalInput")
    d_wp = nc.dram_tensor("wp", [CH, C], BF16, kind="ExternalInput")
    # consts [128, 896]: [tri | zeros | bmask(-3e7*I) | zeros x3 | iden]
    d_consts = nc.dram_tensor("consts", [128, 896], BF16, kind="ExternalInput")
    d_zero8 = nc.dram_tensor("zero8", [128, T], FP8, kind="ExternalInput")
    d_out = nc.dram_tensor("out", [T, C], F32, kind="ExternalOutput")

    with tile.TileContext(nc) as tc, nc.allow_low_precision(
        reason="fp8/bf16 staging; accumulation stays fp32 in PSUM"
    ), (
        tc.tile_pool(name="persist", bufs=1)
    ) as persist, (
        tc.tile_pool(name="pW", bufs=1)
    ) as pW, (
        tc.tile_pool(name="pX", bufs=1)
    ) as pX, (
        tc.tile_pool(name="pO", bufs=3)
    ) as pO, (
        tc.tile_pool(name="p2e", bufs=3)
    ) as p2e, (
        tc.tile_pool(name="p2r", bufs=2)
    ) as p2r, (
        tc.tile_pool(name="p2y", bufs=2)
    ) as p2y, (
        tc.tile_pool(name="psA", bufs=2, space="PSUM")
    ) as psA, (
        tc.tile_pool(name="psS", bufs=1, space="PSUM")
    ) as psS, (
        tc.tile_pool(name="psY", bufs=1, space="PSUM")
    ) as psY:
        # persistent on-chip tensors
        qT8 = [persist.tile([128, 2, T], FP8, tag=f"qT{fo}", name=f"qT{fo}") for fo in range(NFO)]
        kT8 = [persist.tile([128, 2, T], FP8, tag=f"kT{fo}", name=f"kT{fo}") for fo in range(NFO)]
        v = [persist.tile([128, 8, 65], BF16, tag=f"v{i}", name=f"v{i}") for i in range(NTC)]
        yT = persist.tile([128, NFO, T], BF16, tag="yT", name="yT")
        bq_sb = persist.tile([128, NFO], F32, tag="bq")
        bk_sb = persist.tile([128, NFO], F32, tag="bk")
        consts = persist.tile([128, 896], BF16, tag="consts")
        wq8_sb = pW.tile([128, 8, CH], FP8, tag="wq8")
        wk8_sb = pW.tile([128, 8, CH], FP8, tag="wk8")
        wv_sb = pW.tile([128, 8, CH], BF16, tag="wv")
        wp_sb = pW.tile([128, 4, C], BF16, tag="wp")
        x_tiles = [pX.tile([128, 8, 512], BF16, tag=f"x{tq}", name=f"x{tq}") for tq in range(NTQ)]
        x8_tiles = [pX.tile([128, 8, 512], FP8, tag=f"x8{tq}", name=f"x8{tq}") for tq in range(NTQ)]

        tri = consts[:, 0:128]          # tri[j, kt] = (kt > j)
        iden = consts[:, 768:896]
        # bmask rows: c0 diag chunk reads consts[256:256+N] = [-3e7*I | 0...],
        # c1 diag chunk reads consts[128:128+N] = [0 | -3e7*I | 0...]

        def _w_piece(dst, src, c0, c1):
            nc.sync.dma_start(
                dst[:, c0:c1, :],
                src.ap()[128 * c0 : 128 * c1, :].rearrange("(c p) f -> p c f", p=128),
            )

        # input DMAs: first-needed tensors split for a fast first matmul
        nc.sync.dma_start(
            x8_tiles[0][:, 0:2, :],
            d_x8.ap()[0:256, 0:512].rearrange("(c p) t -> p c t", p=128),
        )
        _w_piece(wq8_sb, d_wq8, 0, 2)
        nc.sync.dma_start(bq_sb[:], d_bq.ap())
        nc.sync.dma_start(bk_sb[:], d_bk.ap())
        nc.sync.dma_start(
            x8_tiles[0][:, 2:8, :],
            d_x8.ap()[256:1024, 0:512].rearrange("(c p) t -> p c t", p=128),
        )
        _w_piece(wq8_sb, d_wq8, 2, 8)
        _w_piece(wk8_sb, d_wk8, 0, 8)
        for fo in range(NFO):
            nc.sync.dma_start(qT8[fo][:, 1, :], d_zero8.ap())
            nc.sync.dma_start(kT8[fo][:, 1, :], d_zero8.ap())
        nc.sync.dma_start(consts[:], d_consts.ap())
        nc.sync.dma_start(
            x_tiles[0][:],
            d_xT.ap()[:, 0:512].rearrange("(c p) t -> p c t", p=128),
        )
        _w_piece(wv_sb, d_wv, 0, 8)
        for tq in range(1, NTQ):
            nc.sync.dma_start(
                x8_tiles[tq][:],
                d_x8.ap()[:, 512 * tq : 512 * (tq + 1)].rearrange("(c p) t -> p c t", p=128),
            )
            nc.sync.dma_start(
                x_tiles[tq][:],
                d_xT.ap()[:, 512 * tq : 512 * (tq + 1)].rearrange("(c p) t -> p c t", p=128),
            )
        nc.sync.dma_start(wp_sb[:], d_wp.ap().rearrange("(c p) f -> p c f", p=128))

        def emit_qk_gen(bq_, w_sb, b_sb, dstl, fo):
            # fp8 DoubleRow: 4 matmuls contract 2 ci-subtiles each
            ps = psA.tile([128, 512], F32, tag="psA", name="ps_qk")
            for j in range(4):
                nc.tensor.matmul(
                    ps[:],
                    w_sb[:, 2 * j : 2 * j + 2, 128 * fo : 128 * (fo + 1)],
                    x8_tiles[bq_][:, 2 * j : 2 * j + 2, :],
                    start=(j == 0),
                    stop=(j == 3),
                    perf_mode=DR,
                )
            nc.vector.tensor_scalar(
                dstl[fo][:, 0, 512 * bq_ : 512 * (bq_ + 1)],
                ps[:],
                b_sb[:, fo : fo + 1],
                None,
                op0=ALU.add,
            )

        def emit_v_group(bq_, ts_):
            tci = 4 * bq_ + ts_
            ps = psA.tile([128, 512], F32, tag="psA", name="ps_v")
            for ci in range(8):
                nc.tensor.matmul(
                    ps[:],
                    x_tiles[bq_][:, ci, 128 * ts_ : 128 * (ts_ + 1)],
                    wv_sb[:, ci, :],
                    start=(ci == 0),
                    stop=(ci == 7),
                )
            nc.vector.memset(v[tci][:, :, 64:65], 1.0)
            nc.vector.tensor_copy(
                v[tci][:, :, 0:64],
                ps[:].rearrange("p (h d) -> p h d", h=8),
            )

        def emit_qkv_group(bq_, g):
            if g < 4:
                emit_qk_gen(bq_, wq8_sb, bq_sb, qT8, g)
            elif g < 8:
                emit_qk_gen(bq_, wk8_sb, bk_sb, kT8, g - 4)
            else:
                emit_v_group(bq_, g - 8)

        def emit_proj_tc(tci):
            o_sb = pO.tile([128, C], F32, tag="o", name="o_sb")
            for co in range(2):
                ps = psA.tile([128, 512], F32, tag="psA", name="ps_o")
                for fo in range(NFO):
                    nc.tensor.matmul(
                        ps[:],
                        yT[:, fo, 128 * tci : 128 * (tci + 1)],
                        wp_sb[:, fo, 512 * co : 512 * (co + 1)],
                        start=(fo == 0),
                        stop=(fo == 3),
                    )
                nc.vector.tensor_copy(o_sb[:, 512 * co : 512 * (co + 1)], ps[:])
            nc.sync.dma_start(
                d_out.ap()[128 * tci : 128 * (tci + 1), :],
                o_sb[:],
            )

        def emit_attn_pair(fo, b, p, pyA, pyB):
            """QK pair + mask + exp + AV for chunks (2p, 2p+1) of block b."""
            c0 = 2 * p
            i0 = c0 - 4 * b                      # diag index of first chunk
            qoff = 128 * max(i0, 0)
            q0 = 512 * b
            pS = psS.tile([128, 2048], F32, tag="pS", name="pS")
            for ci, c in enumerate((c0, c0 + 1)):
                is_diag = c >= 4 * b
                for h in range(2):
                    base = 1024 * ci + 512 * h
                    nc.tensor.matmul(
                        pS[:, base + qoff : base + 512],
                        kT8[fo][64 * h : 64 * h + 64, :, 128 * c : 128 * (c + 1)],
                        qT8[fo][64 * h : 64 * h + 64, :, q0 + qoff : q0 + 512],
                        start=True,
                        stop=not is_diag,
                        perf_mode=DR,
                    )
                    if is_diag:
                        # rank-127 mask matmul adds -3e7 above the diagonal of
                        # this chunk's own 128-block; moving operand slides so
                        # the -3e7*I block lands at the right q offset.
                        moff = 256 if c == c0 else 128
                        nc.tensor.matmul(
                            pS[:, base + qoff : base + 512],
                            tri,
                            consts[:, moff : moff + (512 - qoff)],
                            start=False,
                            stop=True,
                        )
            eST = p2e.tile([128, 2048], BF16, tag="eST", name="eST")
            pS4 = pS[:].rearrange("p (c h q) -> p c h q", c=2, h=2)
            eST4 = eST[:].rearrange("p (c h q) -> p c h q", c=2, h=2)
            nc.scalar.activation(
                eST4[:, :, :, qoff:512], pS4[:, :, :, qoff:512], AF.Exp, scale=EXP_SCALE
            )
            # PSUM groups are bank-granular: each head's [128,4,65] bank is ONE
            # accumulation group — start on the first write (qsub0,c0 of pair
            # 0), stop on the last (qsub3, last chunk), pending-zero handles
            # the disjoint qsub regions.
            for qsub in range(4):
                for ci, c in enumerate((c0, c0 + 1)):
                    if c > 4 * b + qsub:
                        continue
                    for h, py in ((0, pyA), (1, pyB)):
                        nc.tensor.matmul(
                            py[:, qsub, :],
                            eST[:, 1024 * ci + 512 * h + 128 * qsub : 1024 * ci + 512 * h + 128 * (qsub + 1)],
                            v[c][:, 2 * fo + h, :],
                            start=(qsub == 0 and c == 0),
                            stop=(qsub == 3 and c == 4 * b + 3),
                        )

        def emit_attn_tail(fo, b, pyA, pyB):
            """normalize (per-partition recip), transpose to yT."""
            q0 = 512 * b
            ys = []
            for h, py in ((0, pyA), (1, pyB)):
                rec = p2r.tile([128, 4, 1], F32, tag="rec", name=f"rec{h}")
                nc.vector.reciprocal(rec[:], py[:, :, 64:65])
                y_sb = p2y.tile([128, 4, 64], BF16, tag="y_sb", name=f"y_sb{h}")
                for qsub in range(4):
                    nc.vector.tensor_scalar(
                        y_sb[:, qsub, :],
                        py[:, qsub, 0:64],
                        rec[:, qsub, :],
                        None,
                        op0=ALU.mult,
                    )
                ys.append(y_sb)
            # one PSUM group per partition-half of the bank (h=0: parts 0:64,
            # h=1: parts 64:128); qsub regions share the group via pending-zero
            tp = psA.tile([128, 512], F32, tag="psA", name="tp")
            for qsub in range(4):
                for h in range(2):
                    nc.tensor.matmul(
                        tp[64 * h : 64 * h + 64, 128 * qsub : 128 * (qsub + 1)],
                        ys[h][:, qsub, :],
                        iden,
                        start=(qsub == 0),
                        stop=(qsub == 3),
                    )
            nc.vector.tensor_copy(yT[:, fo, q0 : q0 + 512], tp[:])

        # b=0 QKV first
        for g in range(12):
            emit_qkv_group(0, g)
        for b in range(NTQ):
            npairs = 2 * b + 2
            for fo in range(NFO):
                pyA = psY.tile([128, 4, 65], F32, tag="pyA", name="pyA")
                pyB = psY.tile([128, 4, 65], F32, tag="pyB", name="pyB")
                # PE filler between exp waits: next quarter's QKV groups and
                # projections of finished quarters, one unit per pair-step
                filler = []
                if b < NTQ - 1:
                    filler += [("qkv", b + 1, 3 * fo + i) for i in range(3)]
                if b >= 1:
                    filler += [("proj", 4 * (b - 1) + fo, 0)]
                for p in range(npairs):
                    emit_attn_pair(fo, b, p, pyA, pyB)
                    if p < len(filler):
                        kind, a0, a1 = filler[p]
                        if kind == "qkv":
                            emit_qkv_group(a0, a1)
                        else:
                            emit_proj_tc(a0)
                for it in filler[npairs:]:
                    kind, a0, a1 = it
                    if kind == "qkv":
                        emit_qkv_group(a0, a1)
                    else:
                        emit_proj_tc(a0)
                emit_attn_tail(fo, b, pyA, pyB)
        for tci in range(12, 16):
            emit_proj_tc(tci)

    nc.compile()
    return nc


def _get_nc():
    if "nc" not in _cached:
        _cached["nc"] = _build_nc()
    return _cached["nc"]


E4M3 = ml_dtypes.float8_e4m3fn


def _f8(a):
    return np.clip(np.ascontiguousarray(a, np.float32), -240, 240).astype(E4M3).view(np.uint8)


def _bf(a):
    return np.ascontiguousarray(a, np.float32).astype(ml_dtypes.bfloat16).view(np.uint16)


def kernel(x, W_attn, b_attn, W_proj, b_proj):
    x = np.asarray(x, np.float32)
    W_attn = np.asarray(W_attn, np.float32)
    b_attn = np.asarray(b_attn, np.float32)
    W_proj = np.asarray(W_proj, np.float32)
    b_proj = np.asarray(b_proj, np.float32)

    nc = _get_nc()
    j = np.arange(128)[:, None]
    kt = np.arange(128)[None, :]
    tri = (kt > j).astype(np.float32)            # [128,128]
    consts = np.zeros((128, 896), np.float32)
    consts[:, 0:128] = tri
    consts[:, 256:384] = MBIG * np.eye(128, dtype=np.float32)
    consts[:, 768:896] = np.eye(128, dtype=np.float32)
    consts_u16 = _bf(consts)
    zero8 = np.zeros((128, T), np.uint8)

    in_maps = []
    for c in range(N_CORES):
        b, hh = divmod(c, 2)
        sl = slice(CH * hh, CH * (hh + 1))
        xb = np.ascontiguousarray(x[b].T)
        in_maps.append(
            {
                "xT": _bf(xb),
                "x8": _f8(xb),
                "wq8": _f8(WS * W_attn[:, 0:C][:, sl]),
                "wk8": _f8(WS * W_attn[:, C : 2 * C][:, sl]),
                "wv": _bf(W_attn[:, 2 * C : 3 * C][:, sl]),
                "bq": np.ascontiguousarray(WS * b_attn[0:C][sl].reshape(NFO, 128).T),
                "bk": np.ascontiguousarray(WS * b_attn[C : 2 * C][sl].reshape(NFO, 128).T),
                "wp": _bf(W_proj[sl, :]),
                "consts": consts_u16,
                "zero8": zero8,
            }
        )

    try:
        res = run_bass_kernel_spmd(nc, in_maps, core_ids=list(range(N_CORES)))
    except Exception:
        # transient NRT device wedges happen; one retry is usually enough
        res = run_bass_kernel_spmd(nc, in_maps, core_ids=list(range(N_CORES)))

    bv = b_attn[2 * C : 3 * C]
    const_bias = (bv @ W_proj + b_proj).astype(np.float32)  # [C]
    out = np.empty((B, T, C), np.float32)
    for b in range(B):
        out[b] = res.results[2 * b]["out"] + res.results[2 * b + 1]["out"] + const_bias
    return out


# revision 12
# speedup vs baseline: 1.3896x; 1.3896x over previous
"""Causal self-attention on 8 TRN2 NeuronCores — v2.

Reference computation (B=4, T=2048, C=1024, H=16, D=64, fp32):
    qkv = x @ W_attn + b_attn ; split q,k,v ; per-head causal softmax(q k^T / 8) @ v
    y = heads @ W_proj + b_proj

Sharding: core c handles batch b = c//2 and head-half hh = c%2 (8 heads).
QKV weights column-split, W_proj row-split; host sums the two partial
projections per batch and adds the constant bias (b_proj + b_v @ W_proj,
exact because softmax rows sum to 1). No collectives.

v2 design notes (cost-model driven; matmul cost = out_free x cycles/row,
fp8e4+DoubleRow = 0.5 cycles/row regardless of contraction size):
  - q,k generated with fp8 DoubleRow matmuls (x8/wq8/wk8 fp8, weights
    pre-scaled x32 so W~0.02 escapes fp8 subnormals; biases x32 to match;
    the 1/1024 descale is folded into the exp scale).
  - v generated in bf16 (value path dominates the error budget).
  - QK^T uses "padded DoubleRow": per head the contraction is d=64, so
    operands are [64 partitions, 2 k-subtiles, N] with subtile 1 an
    all-zero plane (host-DMA'd zeros) -> 0.5 cycles/row at K=64.
  - Causal mask is folded into PSUM before exp via a rank-127 constant
    bf16 matmul that adds -3e7 above the diagonal; exp then flushes those
    to 0. No DVE masking anywhere.
  - exp runs on chunk PAIRS ([128, 2048] ACT ops over a 4-bank PSUM tile)
    to amortize the ACT access-latency; per-chunk q-trimming via a 4D AP.
  - AV is computed in the SWAPPED layout: stationary = exp-scores chunk
    [kt, q], moving = v_aug [kt, 64 v | 1] -> out y[q, d|den] with free
    size 65 (half the cost of the yT layout), denominator per PARTITION.
    Normalize is then a cheap per-partition tensor_scalar; yT for the
    projection is rebuilt with identity matmul transposes (head B lands
    at partition offset 64 via tile_position).
  - PSUM→SBUF traffic and copies are spread across DVE and Pool (GPSIMD)
    to keep everything under the ACT exp floor (~131 us), the binding
    engine for this kernel.
"""

import numpy as np
import ml_dtypes

import concourse.bacc as bacc
import concourse.mybir as mybir
import concourse.tile as tile
from concourse.bass_utils import run_bass_kernel_spmd

F32 = mybir.dt.float32
BF16 = mybir.dt.bfloat16
FP8 = mybir.dt.float8e4
AF = mybir.ActivationFunctionType
ALU = mybir.AluOpType
DR = mybir.MatmulPerfMode.DoubleRow

N_CORES = 8
B, T, C = 4, 2048, 1024
H, D = 16, 64
CH = 512            # features per core (8 heads * 64)
NFO = 4             # head-pair chunks of 128 features
NTQ = 4             # t quarters of 512
NTC = 16            # t chunks of 128
WS = 32.0           # host-side weight prescale for fp8
EXP_SCALE = 0.125 / (WS * WS)   # softmax 1/8, undoing q,k x32 scales
MBIG = -3.0e7       # causal mask addend (pre-scale): exp -> 0

_cached = {}


def _build_nc():
    nc = bacc.Bacc("TRN2", debug=False, num_devices=N_CORES)

    d_xT = nc.dram_tensor("xT", [C, T], BF16, kind="ExternalInput")
    d_x8 = nc.dram_tensor("x8", [C, T], FP8, kind="ExternalInput")
    d_wq8 = nc.dram_tensor("wq8", [C, CH], FP8, kind="ExternalInput")
    d_wk8 = nc.dram_tensor("wk8", [C, CH], FP8, kind="ExternalInput")
    d_wv = nc.dram_tensor("wv", [C, CH], BF16, kind="ExternalInput")
    d_bq = nc.dram_tensor("bq", [128, NFO], F32, kind="ExternalInput")
    d_bk = nc.dram_tensor("bk", [128, NFO], F32, kind="ExternalInput")
    d_wp = nc.dram_tensor("wp", [CH, C], BF16, kind="ExternalInput")
    # consts [128, 896]: [tri | zeros | bmask(-3e7*I) | zeros x3 | iden]
    d_consts = nc.dram_tensor("consts", [128, 896], BF16, kind="ExternalInput")
    d_zero8 = nc.dram_tensor("zero8", [128, T], FP8, kind="ExternalInput")
    d_out = nc.dram_tensor("out", [T, C], F32, kind="ExternalOutput")

    with tile.TileContext(nc) as tc, nc.allow_low_precision(
        reason="fp8/bf16 staging; accumulation stays fp32 in PSUM"
    ), (
        tc.tile_pool(name="persist", bufs=1)
    ) as persist, (
        tc.tile_pool(name="pW", bufs=1)
    ) as pW, (
        tc.tile_pool(name="pX", bufs=1)
    ) as pX, (
        tc.tile_pool(name="pO", bufs=3)
    ) as pO, (
        tc.tile_pool(name="p2e", bufs=3)
    ) as p2e, (
        tc.tile_pool(name="p2r", bufs=2)
    ) as p2r, (
        tc.tile_pool(name="p2y", bufs=2)
    ) as p2y, (
        tc.tile_pool(name="psA", bufs=2, space="PSUM")
    ) as psA, (
        tc.tile_pool(name="psS", bufs=2, space="PSUM")
    ) as psS, (
        tc.tile_pool(name="psY", bufs=1, space="PSUM")
    ) as psY:
        # persistent on-chip tensors
        qT8 = [persist.tile([128, 2, T], FP8, tag=f"qT{fo}", name=f"qT{fo}") for fo in range(NFO)]
        kT8 = [persist.tile([128, 2, T], FP8, tag=f"kT{fo}", name=f"kT{fo}") for fo in range(NFO)]
        v = [persist.tile([128, 8, 65], BF16, tag=f"v{i}", name=f"v{i}") for i in range(NTC)]
        yT = persist.tile([128, NFO, T], BF16, tag="yT", name="yT")
        bq_sb = persist.tile([128, NFO], F32, tag="bq")
        bk_sb = persist.tile([128, NFO], F32, tag="bk")
        consts = persist.tile([128, 896], BF16, tag="consts")
        wq8_sb = pW.tile([128, 8, CH], FP8, tag="wq8")
        wk8_sb = pW.tile([128, 8, CH], FP8, tag="wk8")
        wv_sb = pW.tile([128, 8, CH], BF16, tag="wv")
        wp_sb = pW.tile([128, 4, C], BF16, tag="wp")
        x_tiles = [pX.tile([128, 8, 512], BF16, tag=f"x{tq}", name=f"x{tq}") for tq in range(NTQ)]
        x8_tiles = [pX.tile([128, 8, 512], FP8, tag=f"x8{tq}", name=f"x8{tq}") for tq in range(NTQ)]

        tri = consts[:, 0:128]          # tri[j, kt] = (kt > j)
        iden = consts[:, 768:896]
        # bmask rows: c0 diag chunk reads consts[256:256+N] = [-3e7*I | 0...],
        # c1 diag chunk reads consts[128:128+N] = [0 | -3e7*I | 0...]

        def _w_piece(dst, src, c0, c1):
            nc.sync.dma_start(
                dst[:, c0:c1, :],
                src.ap()[128 * c0 : 128 * c1, :].rearrange("(c p) f -> p c f", p=128),
            )

        # input DMAs: first-needed tensors split for a fast first matmul
        nc.sync.dma_start(
            x8_tiles[0][:, 0:2, :],
            d_x8.ap()[0:256, 0:512].rearrange("(c p) t -> p c t", p=128),
        )
        _w_piece(wq8_sb, d_wq8, 0, 2)
        nc.sync.dma_start(bq_sb[:], d_bq.ap())
        nc.sync.dma_start(bk_sb[:], d_bk.ap())
        nc.sync.dma_start(
            x8_tiles[0][:, 2:8, :],
            d_x8.ap()[256:1024, 0:512].rearrange("(c p) t -> p c t", p=128),
        )
        _w_piece(wq8_sb, d_wq8, 2, 8)
        _w_piece(wk8_sb, d_wk8, 0, 8)
        for fo in range(NFO):
            nc.sync.dma_start(qT8[fo][:, 1, :], d_zero8.ap())
            nc.sync.dma_start(kT8[fo][:, 1, :], d_zero8.ap())
        nc.sync.dma_start(consts[:], d_consts.ap())
        nc.sync.dma_start(
            x_tiles[0][:],
            d_xT.ap()[:, 0:512].rearrange("(c p) t -> p c t", p=128),
        )
        _w_piece(wv_sb, d_wv, 0, 8)
        for tq in range(1, NTQ):
            nc.sync.dma_start(
                x8_tiles[tq][:],
                d_x8.ap()[:, 512 * tq : 512 * (tq + 1)].rearrange("(c p) t -> p c t", p=128),
            )
            nc.sync.dma_start(
                x_tiles[tq][:],
                d_xT.ap()[:, 512 * tq : 512 * (tq + 1)].rearrange("(c p) t -> p c t", p=128),
            )
        nc.sync.dma_start(wp_sb[:], d_wp.ap().rearrange("(c p) f -> p c f", p=128))

        def emit_qk_gen(bq_, w_sb, b_sb, dstl, fo):
            # fp8 DoubleRow: 4 matmuls contract 2 ci-subtiles each
            ps = psA.tile([128, 512], F32, tag="psA", name="ps_qk")
            for j in range(4):
                nc.tensor.matmul(
                    ps[:],
                    w_sb[:, 2 * j : 2 * j + 2, 128 * fo : 128 * (fo + 1)],
                    x8_tiles[bq_][:, 2 * j : 2 * j + 2, :],
                    start=(j == 0),
                    stop=(j == 3),
                    perf_mode=DR,
                )
            nc.vector.tensor_scalar(
                dstl[fo][:, 0, 512 * bq_ : 512 * (bq_ + 1)],
                ps[:],
                b_sb[:, fo : fo + 1],
                None,
                op0=ALU.add,
            )

        def emit_v_group(bq_, ts_):
            tci = 4 * bq_ + ts_
            ps = psA.tile([128, 512], F32, tag="psA", name="ps_v")
            for ci in range(8):
                nc.tensor.matmul(
                    ps[:],
                    x_tiles[bq_][:, ci, 128 * ts_ : 128 * (ts_ + 1)],
                    wv_sb[:, ci, :],
                    start=(ci == 0),
                    stop=(ci == 7),
                )
            nc.vector.memset(v[tci][:, :, 64:65], 1.0)
            nc.vector.tensor_copy(
                v[tci][:, :, 0:64],
                ps[:].rearrange("p (h d) -> p h d", h=8),
            )

        def emit_qkv_group(bq_, g):
            if g < 4:
                emit_qk_gen(bq_, wq8_sb, bq_sb, qT8, g)
            elif g < 8:
                emit_qk_gen(bq_, wk8_sb, bk_sb, kT8, g - 4)
            else:
                emit_v_group(bq_, g - 8)

        def emit_proj_tc(tci):
            o_sb = pO.tile([128, C], F32, tag="o", name="o_sb")
            for co in range(2):
                ps = psA.tile([128, 512], F32, tag="psA", name="ps_o")
                for fo in range(NFO):
                    nc.tensor.matmul(
                        ps[:],
                        yT[:, fo, 128 * tci : 128 * (tci + 1)],
                        wp_sb[:, fo, 512 * co : 512 * (co + 1)],
                        start=(fo == 0),
                        stop=(fo == 3),
                    )
                nc.vector.tensor_copy(o_sb[:, 512 * co : 512 * (co + 1)], ps[:])
            nc.sync.dma_start(
                d_out.ap()[128 * tci : 128 * (tci + 1), :],
                o_sb[:],
            )

        def emit_attn_chunk(fo, b, c, pyA, pyB):
            """QK + mask + exp + AV for k-chunk c (128 keys) of block b."""
            i = c - 4 * b                        # diag index (>=0: diagonal)
            qoff = 128 * max(i, 0)
            q0 = 512 * b
            pS = psS.tile([128, 1024], F32, tag="pS", name="pS")
            for h in range(2):
                nc.tensor.matmul(
                    pS[:, 512 * h + qoff : 512 * h + 512],
                    kT8[fo][64 * h : 64 * h + 64, :, 128 * c : 128 * (c + 1)],
                    qT8[fo][64 * h : 64 * h + 64, :, q0 + qoff : q0 + 512],
                    start=True,
                    stop=(i < 0),
                    perf_mode=DR,
                )
                if i >= 0:
                    # rank-127 mask matmul adds -3e7 above the diagonal of
                    # this chunk's own 128-wide q block
                    nc.tensor.matmul(
                        pS[:, 512 * h + qoff : 512 * h + qoff + 128],
                        tri,
                        consts[:, 256:384],
                        start=False,
                        stop=True,
                        skip_group_check=True,
                    )
            eST = p2e.tile([128, 1024], BF16, tag="eST", name="eST")
            pS3 = pS[:].rearrange("p (h q) -> p h q", h=2)
            eST3 = eST[:].rearrange("p (h q) -> p h q", h=2)
            nc.scalar.activation(
                eST3[:, :, qoff:512], pS3[:, :, qoff:512], AF.Exp, scale=EXP_SCALE
            )
            # PSUM groups are bank-granular: each head's [128,4,65] bank is ONE
            # accumulation group — start on the first write (qsub0 of chunk 0),
            # stop on the last (qsub3 of the final chunk); pending-zero handles
            # the disjoint qsub regions.
            for qsub in range(max(i, 0), 4):
                for h, py in ((0, pyA), (1, pyB)):
                    nc.tensor.matmul(
                        py[:, qsub, :],
                        eST[:, 512 * h + 128 * qsub : 512 * h + 128 * (qsub + 1)],
                        v[c][:, 2 * fo + h, :],
                        start=(qsub == 0 and c == 0),
                        stop=(qsub == 3 and c == 4 * b + 3),
                    )

        def emit_attn_tail(fo, b, pyA, pyB):
            """normalize (per-partition recip), transpose to yT."""
            q0 = 512 * b
            ys = []
            for h, py in ((0, pyA), (1, pyB)):
                rec = p2r.tile([128, 4, 1], F32, tag="rec", name=f"rec{h}")
                nc.vector.reciprocal(rec[:], py[:, :, 64:65])
                y_sb = p2y.tile([128, 4, 64], BF16, tag="y_sb", name=f"y_sb{h}")
                for qsub in range(4):
                    nc.vector.tensor_scalar(
                        y_sb[:, qsub, :],
                        py[:, qsub, 0:64],
                        rec[:, qsub, :],
                        None,
                        op0=ALU.mult,
                    )
                ys.append(y_sb)
            # one PSUM group per partition-half of the bank (h=0: parts 0:64,
            # h=1: parts 64:128); qsub regions share the group via pending-zero
            tp = psA.tile([128, 512], F32, tag="psA", name="tp")
            for qsub in range(4):
                for h in range(2):
                    nc.tensor.matmul(
                        tp[64 * h : 64 * h + 64, 128 * qsub : 128 * (qsub + 1)],
                        ys[h][:, qsub, :],
                        iden,
                        start=(qsub == 0),
                        stop=(qsub == 3),
                    )
            nc.vector.tensor_copy(yT[:, fo, q0 : q0 + 512], tp[:])

        # b=0 QKV first
        for g in range(12):
            emit_qkv_group(0, g)
        for b in range(NTQ):
            nchunks = 4 * b + 4
            for fo in range(NFO):
                pyA = psY.tile([128, 4, 65], F32, tag="pyA", name="pyA")
                pyB = psY.tile([128, 4, 65], F32, tag="pyB", name="pyB")
                # PE filler between exp waits: next quarter's QKV groups and
                # projections of finished quarters, one unit per chunk-step
                filler = []
                if b < NTQ - 1:
                    filler += [("qkv", b + 1, 3 * fo + i) for i in range(3)]
                if b >= 1:
                    filler += [("proj", 4 * (b - 1) + fo, 0)]
                # spread filler units across the chunk steps
                slots = {}
                for fi, it in enumerate(filler):
                    slots[(fi + 1) * nchunks // (len(filler) + 1)] = it
                for c in range(nchunks):
                    emit_attn_chunk(fo, b, c, pyA, pyB)
                    it = slots.get(c)
                    if it is not None:
                        kind, a0, a1 = it
                        if kind == "qkv":
                            emit_qkv_group(a0, a1)
                        else:
                            emit_proj_tc(a0)
                emit_attn_tail(fo, b, pyA, pyB)
        for tci in range(12, 16):
            emit_proj_tc(tci)

    nc.compile()
    return nc


def _get_nc():
    if "nc" not in _cached:
        _cached["nc"] = _build_nc()
    return _cached["nc"]


E4M3 = ml_dtypes.float8_e4m3fn


def _f8(a):
    return np.clip(np.ascontiguousarray(a, np.float32), -240, 240).astype(E4M3).view(np.uint8)


def _bf(a):
    return np.ascontiguousarray(a, np.float32).astype(ml_dtypes.bfloat16).view(np.uint16)


def kernel(x, W_attn, b_attn, W_proj, b_proj):
    x = np.asarray(x, np.float32)
    W_attn = np.asarray(W_attn, np.float32)
    b_attn = np.asarray(b_attn, np.float32)
    W_proj = np.asarray(W_proj, np.float32)
    b_proj = np.asarray(b_proj, np.float32)

    nc = _get_nc()
    j = np.arange(128)[:, None]
    kt = np.arange(128)[None, :]
    tri = (kt > j).astype(np.float32)            # [128,128]
    consts = np.zeros((128, 896), np.float32)
    consts[:, 0:128] = tri
    consts[:, 256:384] = MBIG * np.eye(128, dtype=np.float32)
    consts[:, 768:896] = np.eye(128, dtype=np.float32)
    consts_u16 = _bf(consts)
    zero8 = np.zeros((128, T), np.uint8)

    in_maps = []
    for c in range(N_CORES):
        b, hh = divmod(c, 2)
        sl = slice(CH * hh, CH * (hh + 1))
        xb = np.ascontiguousarray(x[b].T)
        in_maps.append(
            {
                "xT": _bf(xb),
                "x8": _f8(xb),
                "wq8": _f8(WS * W_attn[:, 0:C][:, sl]),
                "wk8": _f8(WS * W_attn[:, C : 2 * C][:, sl]),
                "wv": _bf(W_attn[:, 2 * C : 3 * C][:, sl]),
                "bq": np.ascontiguousarray(WS * b_attn[0:C][sl].reshape(NFO, 128).T),
                "bk": np.ascontiguousarray(WS * b_attn[C : 2 * C][sl].reshape(NFO, 128).T),
                "wp": _bf(W_proj[sl, :]),
                "consts": consts_u16,
                "zero8": zero8,
            }
        )

    try:
        res = run_bass_kernel_spmd(nc, in_maps, core_ids=list(range(N_CORES)))
    except Exception:
        # transient NRT device wedges happen; one retry is usually enough
        res = run_bass_kernel_spmd(nc, in_maps, core_ids=list(range(N_CORES)))

    bv = b_attn[2 * C : 3 * C]
    const_bias = (bv @ W_proj + b_proj).astype(np.float32)  # [C]
    out = np.empty((B, T, C), np.float32)
    for b in range(B):
        out[b] = res.results[2 * b]["out"] + res.results[2 * b + 1]["out"] + const_bias
    return out


# revision 16
# speedup vs baseline: 1.4284x; 1.0279x over previous
"""Causal self-attention on 8 TRN2 NeuronCores — v2.

Reference computation (B=4, T=2048, C=1024, H=16, D=64, fp32):
    qkv = x @ W_attn + b_attn ; split q,k,v ; per-head causal softmax(q k^T / 8) @ v
    y = heads @ W_proj + b_proj

Sharding: core c handles batch b = c//2 and head-half hh = c%2 (8 heads).
QKV weights column-split, W_proj row-split; host sums the two partial
projections per batch and adds the constant bias (b_proj + b_v @ W_proj,
exact because softmax rows sum to 1). No collectives.

v2 design notes (cost-model driven; matmul cost = out_free x cycles/row,
fp8e4+DoubleRow = 0.5 cycles/row regardless of contraction size):
  - q,k generated with fp8 DoubleRow matmuls (x8/wq8/wk8 fp8, weights
    pre-scaled x32 so W~0.02 escapes fp8 subnormals; biases x32 to match;
    the 1/1024 descale is folded into the exp scale).
  - v generated in bf16 (value path dominates the error budget).
  - QK^T uses "padded DoubleRow": per head the contraction is d=64, so
    operands are [64 partitions, 2 k-subtiles, N] with subtile 1 an
    all-zero plane (host-DMA'd zeros) -> 0.5 cycles/row at K=64.
  - Causal mask is folded into PSUM before exp via a rank-127 constant
    bf16 matmul that adds -3e7 above the diagonal; exp then flushes those
    to 0. No DVE masking anywhere.
  - exp runs on chunk PAIRS ([128, 2048] ACT ops over a 4-bank PSUM tile)
    to amortize the ACT access-latency; per-chunk q-trimming via a 4D AP.
  - AV is computed in the SWAPPED layout: stationary = exp-scores chunk
    [kt, q], moving = v_aug [kt, 64 v | 1] -> out y[q, d|den] with free
    size 65 (half the cost of the yT layout), denominator per PARTITION.
    Normalize is then a cheap per-partition tensor_scalar; yT for the
    projection is rebuilt with identity matmul transposes (head B lands
    at partition offset 64 via tile_position).
  - PSUM→SBUF traffic and copies are spread across DVE and Pool (GPSIMD)
    to keep everything under the ACT exp floor (~131 us), the binding
    engine for this kernel.
"""

import numpy as np
import ml_dtypes

import concourse.bacc as bacc
import concourse.mybir as mybir
import concourse.tile as tile
from concourse.bass_utils import run_bass_kernel_spmd

F32 = mybir.dt.float32
BF16 = mybir.dt.bfloat16
FP8 = mybir.dt.float8e4
AF = mybir.ActivationFunctionType
ALU = mybir.AluOpType
DR = mybir.MatmulPerfMode.DoubleRow

N_CORES = 8
B, T, C = 4, 2048, 1024
H, D = 16, 64
CH = 512            # features per core (8 heads * 64)
NFO = 4             # head-pair chunks of 128 features
NTQ = 4             # t quarters of 512
NTC = 16            # t chunks of 128
WS = 32.0           # host-side weight prescale for fp8
EXP_SCALE = 0.125 / (WS * WS)   # softmax 1/8, undoing q,k x32 scales
MBIG = -3.0e7       # causal mask addend (pre-scale): exp -> 0

_cached = {}


def _build_nc():
    nc = bacc.Bacc("TRN2", debug=False, num_devices=N_CORES)

    d_xT = nc.dram_tensor("xT", [C, T], BF16, kind="ExternalInput")
    d_x8 = nc.dram_tensor("x8", [C, T], FP8, kind="ExternalInput")
    d_wq8 = nc.dram_tensor("wq8", [C, CH], FP8, kind="ExternalInput")
    d_wk8 = nc.dram_tensor("wk8", [C, CH], FP8, kind="ExternalInput")
    d_wv = nc.dram_tensor("wv", [C, CH], BF16, kind="ExternalInput")
    d_bq = nc.dram_tensor("bq", [128, NFO], F32, kind="ExternalInput")
    d_bk = nc.dram_tensor("bk", [128, NFO], F32, kind="ExternalInput")
    d_wp = nc.dram_tensor("wp", [CH, C], BF16, kind="ExternalInput")
    # consts [128, 896]: [tri | zeros | bmask(-3e7*I) | zeros x3 | iden]
    d_consts = nc.dram_tensor("consts", [128, 896], BF16, kind="ExternalInput")
    d_zero8 = nc.dram_tensor("zero8", [128, T], FP8, kind="ExternalInput")
    d_out = nc.dram_tensor("out", [T, C], F32, kind="ExternalOutput")

    with tile.TileContext(nc) as tc, nc.allow_low_precision(
        reason="fp8/bf16 staging; accumulation stays fp32 in PSUM"
    ), (
        tc.tile_pool(name="persist", bufs=1)
    ) as persist, (
        tc.tile_pool(name="pW", bufs=1)
    ) as pW, (
        tc.tile_pool(name="pX", bufs=1)
    ) as pX, (
        tc.tile_pool(name="pO", bufs=3)
    ) as pO, (
        tc.tile_pool(name="p2e", bufs=3)
    ) as p2e, (
        tc.tile_pool(name="p2r", bufs=2)
    ) as p2r, (
        tc.tile_pool(name="p2y", bufs=2)
    ) as p2y, (
        tc.tile_pool(name="psA", bufs=2, space="PSUM")
    ) as psA, (
        tc.tile_pool(name="psS", bufs=2, space="PSUM")
    ) as psS, (
        tc.tile_pool(name="psY", bufs=1, space="PSUM")
    ) as psY:
        # persistent on-chip tensors
        qT8 = [persist.tile([128, 2, T], FP8, tag=f"qT{fo}", name=f"qT{fo}") for fo in range(NFO)]
        kT8 = [persist.tile([128, 2, T], FP8, tag=f"kT{fo}", name=f"kT{fo}") for fo in range(NFO)]
        v = [persist.tile([128, 8, 65], BF16, tag=f"v{i}", name=f"v{i}") for i in range(NTC)]
        yT = persist.tile([128, NFO, T], BF16, tag="yT", name="yT")
        bq_sb = persist.tile([128, NFO], F32, tag="bq")
        bk_sb = persist.tile([128, NFO], F32, tag="bk")
        consts = persist.tile([128, 896], BF16, tag="consts")
        wq8_sb = pW.tile([128, 8, CH], FP8, tag="wq8")
        wk8_sb = pW.tile([128, 8, CH], FP8, tag="wk8")
        wv_sb = pW.tile([128, 8, CH], BF16, tag="wv")
        wp_sb = pW.tile([128, 4, C], BF16, tag="wp")
        x_tiles = [pX.tile([128, 8, 512], BF16, tag=f"x{tq}", name=f"x{tq}") for tq in range(NTQ)]
        x8_tiles = [pX.tile([128, 8, 512], FP8, tag=f"x8{tq}", name=f"x8{tq}") for tq in range(NTQ)]

        tri = consts[:, 0:128]          # tri[j, kt] = (kt > j)
        iden = consts[:, 768:896]
        # bmask rows: c0 diag chunk reads consts[256:256+N] = [-3e7*I | 0...],
        # c1 diag chunk reads consts[128:128+N] = [0 | -3e7*I | 0...]

        def _w_piece(dst, src, c0, c1):
            nc.sync.dma_start(
                dst[:, c0:c1, :],
                src.ap()[128 * c0 : 128 * c1, :].rearrange("(c p) f -> p c f", p=128),
            )

        # input DMAs: first-needed tensors split for a fast first matmul
        nc.sync.dma_start(
            x8_tiles[0][:, 0:4, :],
            d_x8.ap()[0:512, 0:512].rearrange("(c p) t -> p c t", p=128),
        )
        _w_piece(wq8_sb, d_wq8, 0, 4)
        nc.sync.dma_start(bq_sb[:], d_bq.ap())
        nc.sync.dma_start(bk_sb[:], d_bk.ap())
        nc.sync.dma_start(
            x8_tiles[0][:, 4:8, :],
            d_x8.ap()[512:1024, 0:512].rearrange("(c p) t -> p c t", p=128),
        )
        _w_piece(wq8_sb, d_wq8, 4, 8)
        nc.sync.dma_start(qT8[0][:, 1, :], d_zero8.ap())
        _w_piece(wk8_sb, d_wk8, 0, 8)
        nc.sync.dma_start(kT8[0][:, 1, :], d_zero8.ap())
        nc.sync.dma_start(consts[:], d_consts.ap())
        nc.sync.dma_start(
            x_tiles[0][:],
            d_xT.ap()[:, 0:512].rearrange("(c p) t -> p c t", p=128),
        )
        _w_piece(wv_sb, d_wv, 0, 8)
        for fo in range(1, NFO):
            nc.sync.dma_start(qT8[fo][:, 1, :], d_zero8.ap())
            nc.sync.dma_start(kT8[fo][:, 1, :], d_zero8.ap())
        for tq in range(1, NTQ):
            nc.sync.dma_start(
                x8_tiles[tq][:],
                d_x8.ap()[:, 512 * tq : 512 * (tq + 1)].rearrange("(c p) t -> p c t", p=128),
            )
            nc.sync.dma_start(
                x_tiles[tq][:],
                d_xT.ap()[:, 512 * tq : 512 * (tq + 1)].rearrange("(c p) t -> p c t", p=128),
            )
        nc.sync.dma_start(wp_sb[:], d_wp.ap().rearrange("(c p) f -> p c f", p=128))

        def emit_qk_gen(bq_, w_sb, b_sb, dstl, fo):
            # fp8 DoubleRow: 4 matmuls contract 2 ci-subtiles each
            ps = psA.tile([128, 512], F32, tag="psA", name="ps_qk")
            for j in range(4):
                nc.tensor.matmul(
                    ps[:],
                    w_sb[:, 2 * j : 2 * j + 2, 128 * fo : 128 * (fo + 1)],
                    x8_tiles[bq_][:, 2 * j : 2 * j + 2, :],
                    start=(j == 0),
                    stop=(j == 3),
                    perf_mode=DR,
                )
            nc.vector.tensor_scalar(
                dstl[fo][:, 0, 512 * bq_ : 512 * (bq_ + 1)],
                ps[:],
                b_sb[:, fo : fo + 1],
                None,
                op0=ALU.add,
            )

        def emit_v_group(bq_, ts_):
            tci = 4 * bq_ + ts_
            ps = psA.tile([128, 512], F32, tag="psA", name="ps_v")
            for ci in range(8):
                nc.tensor.matmul(
                    ps[:],
                    x_tiles[bq_][:, ci, 128 * ts_ : 128 * (ts_ + 1)],
                    wv_sb[:, ci, :],
                    start=(ci == 0),
                    stop=(ci == 7),
                )
            nc.vector.memset(v[tci][:, :, 64:65], 1.0)
            nc.vector.tensor_copy(
                v[tci][:, :, 0:64],
                ps[:].rearrange("p (h d) -> p h d", h=8),
            )

        def emit_qkv_group(bq_, g):
            if g < 4:
                emit_qk_gen(bq_, wq8_sb, bq_sb, qT8, g)
            elif g < 8:
                emit_qk_gen(bq_, wk8_sb, bk_sb, kT8, g - 4)
            else:
                emit_v_group(bq_, g - 8)

        def emit_proj_tc(tci):
            o_sb = pO.tile([128, C], F32, tag="o", name="o_sb")
            for co in range(2):
                ps = psA.tile([128, 512], F32, tag="psA", name="ps_o")
                for fo in range(NFO):
                    nc.tensor.matmul(
                        ps[:],
                        yT[:, fo, 128 * tci : 128 * (tci + 1)],
                        wp_sb[:, fo, 512 * co : 512 * (co + 1)],
                        start=(fo == 0),
                        stop=(fo == 3),
                    )
                nc.vector.tensor_copy(o_sb[:, 512 * co : 512 * (co + 1)], ps[:])
                nc.sync.dma_start(
                    d_out.ap()[128 * tci : 128 * (tci + 1), 512 * co : 512 * (co + 1)],
                    o_sb[:, 512 * co : 512 * (co + 1)],
                )

        def emit_attn_chunk(fo, b, c, pyA, pyB):
            """QK + mask + exp + AV for k-chunk c (128 keys) of block b."""
            i = c - 4 * b                        # diag index (>=0: diagonal)
            qoff = 128 * max(i, 0)
            q0 = 512 * b
            pS = psS.tile([128, 1024], F32, tag="pS", name="pS")
            for h in range(2):
                nc.tensor.matmul(
                    pS[:, 512 * h + qoff : 512 * h + 512],
                    kT8[fo][64 * h : 64 * h + 64, :, 128 * c : 128 * (c + 1)],
                    qT8[fo][64 * h : 64 * h + 64, :, q0 + qoff : q0 + 512],
                    start=True,
                    stop=(i < 0),
                    perf_mode=DR,
                )
                if i >= 0:
                    # rank-127 mask matmul adds -3e7 above the diagonal of
                    # this chunk's own 128-wide q block
                    nc.tensor.matmul(
                        pS[:, 512 * h + qoff : 512 * h + qoff + 128],
                        tri,
                        consts[:, 256:384],
                        start=False,
                        stop=True,
                        skip_group_check=True,
                    )
            eST = p2e.tile([128, 1024], BF16, tag="eST", name="eST")
            pS3 = pS[:].rearrange("p (h q) -> p h q", h=2)
            eST3 = eST[:].rearrange("p (h q) -> p h q", h=2)
            nc.scalar.activation(
                eST3[:, :, qoff:512], pS3[:, :, qoff:512], AF.Exp, scale=EXP_SCALE
            )
            # PSUM groups are bank-granular: each head's [128,4,65] bank is ONE
            # accumulation group — start on the first write (qsub0 of chunk 0),
            # stop on the last (qsub3 of the final chunk); pending-zero handles
            # the disjoint qsub regions.
            for qsub in range(max(i, 0), 4):
                for h, py in ((0, pyA), (1, pyB)):
                    nc.tensor.matmul(
                        py[:, qsub, :],
                        eST[:, 512 * h + 128 * qsub : 512 * h + 128 * (qsub + 1)],
                        v[c][:, 2 * fo + h, :],
                        start=(qsub == 0 and c == 0),
                        stop=(qsub == 3 and c == 4 * b + 3),
                    )

        def emit_attn_tail(fo, b, pyA, pyB):
            """normalize (per-partition recip), transpose to yT."""
            q0 = 512 * b
            ys = []
            for h, py in ((0, pyA), (1, pyB)):
                rec = p2r.tile([128, 4, 1], F32, tag="rec", name=f"rec{h}")
                nc.vector.reciprocal(rec[:], py[:, :, 64:65])
                y_sb = p2y.tile([128, 4, 64], BF16, tag="y_sb", name=f"y_sb{h}")
                for qsub in range(4):
                    nc.vector.tensor_scalar(
                        y_sb[:, qsub, :],
                        py[:, qsub, 0:64],
                        rec[:, qsub, :],
                        None,
                        op0=ALU.mult,
                    )
                ys.append(y_sb)
            # one PSUM group per partition-half of the bank (h=0: parts 0:64,
            # h=1: parts 64:128); qsub regions share the group via pending-zero
            tp = psA.tile([128, 512], F32, tag="psA", name="tp")
            for qsub in range(4):
                for h in range(2):
                    nc.tensor.matmul(
                        tp[64 * h : 64 * h + 64, 128 * qsub : 128 * (qsub + 1)],
                        ys[h][:, qsub, :],
                        iden,
                        start=(qsub == 0),
                        stop=(qsub == 3),
                    )
            nc.vector.tensor_copy(yT[:, fo, q0 : q0 + 512], tp[:])

        # Filler scheduling: PE work (QKV gen of the NEXT quarter, projections
        # of finished quarters) dripped between attention chunks so the PE
        # stays fed while ACT grinds exp. Emission order = program order, so
        # deadlines are enforced by draining queues before dependent chunks.
        def emit_unit(u):
            if u[0] == "qkv":
                emit_qkv_group(u[1], u[2])
            else:
                emit_proj_tc(u[1])

        def drain_until(queue, need):
            while need & {(u[1], u[2]) for u in queue if u[0] == "qkv"}:
                emit_unit(queue.pop(0))

        GEN_ORDER = (8, 9, 10, 11, 0, 4, 1, 5, 2, 6, 3, 7)  # v first, then q,k by fo
        # b=0 prefix: only what the first attention chunk needs
        emit_qkv_group(0, 0)
        emit_qkv_group(0, 4)
        emit_qkv_group(0, 8)
        fillers = {
            0: [("qkv", 0, g) for g in (9, 10, 11, 1, 5, 2, 6, 3, 7)]
               + [("qkv", 1, g) for g in GEN_ORDER],
            1: [("qkv", 2, g) for g in GEN_ORDER],
            2: [("qkv", 3, g) for g in GEN_ORDER] + [("proj", t) for t in range(4)],
            3: [("proj", t) for t in range(4, 12)],
        }
        for b in range(NTQ):
            nchunks = 4 * b + 4
            queue = fillers[b]
            q0len = len(queue)
            nslots = NFO * nchunks
            slot = 0
            for fo in range(NFO):
                if b == 0 and fo > 0:
                    drain_until(queue, {(0, fo), (0, 4 + fo)})
                pyA = psY.tile([128, 4, 65], F32, tag="pyA", name="pyA")
                pyB = psY.tile([128, 4, 65], F32, tag="pyB", name="pyB")
                for c in range(nchunks):
                    if b == 0 and fo == 0 and c > 0:
                        drain_until(queue, {(0, 8 + c)})
                    emit_attn_chunk(fo, b, c, pyA, pyB)
                    slot += 1
                    while queue and (q0len - len(queue)) < slot * q0len // nslots:
                        emit_unit(queue.pop(0))
                emit_attn_tail(fo, b, pyA, pyB)
            while queue:
                emit_unit(queue.pop(0))
        for tci in range(12, 16):
            emit_proj_tc(tci)

    nc.compile()
    return nc


def _get_nc():
    if "nc" not in _cached:
        _cached["nc"] = _build_nc()
    return _cached["nc"]


E4M3 = ml_dtypes.float8_e4m3fn


def _f8(a):
    return np.clip(np.ascontiguousarray(a, np.float32), -240, 240).astype(E4M3).view(np.uint8)


def _bf(a):
    return np.ascontiguousarray(a, np.float32).astype(ml_dtypes.bfloat16).view(np.uint16)


def kernel(x, W_attn, b_attn, W_proj, b_proj):
    x = np.asarray(x, np.float32)
    W_attn = np.asarray(W_attn, np.float32)
    b_attn = np.asarray(b_attn, np.float32)
    W_proj = np.asarray(W_proj, np.float32)
    b_proj = np.asarray(b_proj, np.float32)

    nc = _get_nc()
    j = np.arange(128)[:, None]
    kt = np.arange(128)[None, :]
    tri = (kt > j).astype(np.float32)            # [128,128]
    consts = np.zeros((128, 896), np.float32)
    consts[:, 0:128] = tri
    consts[:, 256:384] = MBIG * np.eye(128, dtype=np.float32)
    consts[:, 768:896] = np.eye(128, dtype=np.float32)
    consts_u16 = _bf(consts)
    zero8 = np.zeros((128, T), np.uint8)

    in_maps = []
    for c in range(N_CORES):
        b, hh = divmod(c, 2)
        sl = slice(CH * hh, CH * (hh + 1))
        xb = np.ascontiguousarray(x[b].T)
        in_maps.append(
            {
                "xT": _bf(xb),
                "x8": _f8(xb),
                "wq8": _f8(WS * W_attn[:, 0:C][:, sl]),
                "wk8": _f8(WS * W_attn[:, C : 2 * C][:, sl]),
                "wv": _bf(W_attn[:, 2 * C : 3 * C][:, sl]),
                "bq": np.ascontiguousarray(WS * b_attn[0:C][sl].reshape(NFO, 128).T),
                "bk": np.ascontiguousarray(WS * b_attn[C : 2 * C][sl].reshape(NFO, 128).T),
                "wp": _bf(W_proj[sl, :]),
                "consts": consts_u16,
                "zero8": zero8,
            }
        )

    try:
        res = run_bass_kernel_spmd(nc, in_maps, core_ids=list(range(N_CORES)))
    except Exception:
        # transient NRT device wedges happen; one retry is usually enough
        res = run_bass_kernel_spmd(nc, in_maps, core_ids=list(range(N_CORES)))

    bv = b_attn[2 * C : 3 * C]
    const_bias = (bv @ W_proj + b_proj).astype(np.float32)  # [C]
    out = np.empty((B, T, C), np.float32)
    for b in range(B):
        out[b] = res.results[2 * b]["out"] + res.results[2 * b + 1]["out"] + const_bias
    return out
